# revision 1
# baseline (speedup 1.0000x reference)
"""DeepBKT 4-layer transformer forward on 8 TRN2 NeuronCores.

Data-parallel over batch: B=16 -> 2 batch items per core; each core runs the
full 4-layer stack on its (2*512, 512) token slab.

Layout strategy per core (T = 1024 tokens, D = 512):
  - residual stream x kept in NORMAL layout (tokens on partitions, 8 tiles of
    (128, 512) fp32) so LayerNorm row-stats use free-dim accumulators.
  - matmul operands kept in TRANSPOSED layout (feature dim on partitions,
    tokens on free) as float32r (full-rate PE, ~1e-4 matmul rel err).
  - attention in transposed-score layout: forget_rate (and 1/sqrt(dk)) is
    folded into the *query* copy of the shared q/k projection, so
    scoresT(k,q) needs no per-column scaling; exp runs directly on the gram
    psum; softmax denominators come from a ones-column appended to V (row 64
    of the AV psum).  Strict causality is block-structured: only the 128x128
    diagonal block needs a multiplicative {0,1} mask on exp(scores); key
    blocks beyond the query tile are never computed.  Row 0 of each batch
    item gets an all-zero attention row (matches the reference's zero_pad).
"""

import numpy as np

import concourse.bass as bass
import concourse.tile as tile
from concourse import bacc, mybir
from concourse.bass_utils import run_bass_kernel_spmd

F32 = mybir.dt.float32
R = mybir.dt.float32r
AF = mybir.ActivationFunctionType
OP = mybir.AluOpType

B, S, D, H, DFF, L = 16, 512, 512, 8, 2048, 4
DK = D // H                       # 64
NCORES = 8
BPC = B // NCORES                 # 2 batch items per core
T = BPC * S                       # 1024 tokens per core
NT = T // 128                     # 8 token tiles
NC = D // 128                     # 4 feature chunks
NF = DFF // 128                   # 16 ffn chunks
EPS = 1e-5
SCALE = 1.0 / np.sqrt(DK)

_CACHE = {}


def _build(flags):
    use_bk, use_bv, use_bo, use_b1, use_b2, use_ln1, use_ln2 = flags

    nc = bacc.Bacc("TRN2", target_bir_lowering=False, debug=False,
                   num_devices=NCORES)

    d = {}
    d["x0_d"] = nc.dram_tensor("x0", [T, D], F32, kind="ExternalInput")
    d["y0_d"] = nc.dram_tensor("y0", [T, D], F32, kind="ExternalInput")
    d["frs_d"] = nc.dram_tensor("frs", [128, T], F32, kind="ExternalInput")
    d["mask_d"] = nc.dram_tensor("mask01", [128, 128], R, kind="ExternalInput")
    d["wk_d"] = nc.dram_tensor("Wk", [L, D, D], R, kind="ExternalInput")
    d["wv_d"] = nc.dram_tensor("Wv", [L, D, D], R, kind="ExternalInput")
    d["wo_d"] = nc.dram_tensor("Wo", [L, D, D], R, kind="ExternalInput")
    d["w1_d"] = nc.dram_tensor("W1", [L, D, DFF], R, kind="ExternalInput")
    d["w2_d"] = nc.dram_tensor("W2", [L, DFF, D], R, kind="ExternalInput")
    d["bk_d"] = nc.dram_tensor("bk", [L, D, 1], F32, kind="ExternalInput")
    d["b1_d"] = nc.dram_tensor("b1", [L, DFF, 1], F32, kind="ExternalInput")
    # free-axis vectors pre-broadcast on host to (128, D)
    for nm in ("bvb", "bob", "b2b", "g1b", "be1b", "g2b", "be2b"):
        d[nm + "_d"] = nc.dram_tensor(nm, [L, 128, D], F32, kind="ExternalInput")
    d["out_d"] = nc.dram_tensor("out", [T, D], F32, kind="ExternalOutput")

    with tile.TileContext(nc) as tc:
        _emit(nc, tc, d, flags)
    nc.compile()
    return nc


def _emit(nc, tc, d, flags):
    use_bk, use_bv, use_bo, use_b1, use_b2, use_ln1, use_ln2 = flags
    import contextlib
    ctx = contextlib.ExitStack()
    with ctx:
        sb = ctx.enter_context(tc.tile_pool(name="sb", bufs=1))
        ps = ctx.enter_context(tc.tile_pool(name="ps", bufs=4, space="PSUM"))

        def tl(shape, dtype, tag, bufs, name=None):
            return sb.tile(shape, dtype, tag=tag, bufs=bufs, name=name or tag)

        def pA():
            return ps.tile([128, 512], F32, tag="psA", bufs=4, name="psA")

        def pB():
            return ps.tile([128, 512], F32, tag="psB", bufs=4, name="psB")

        # ---- constants ----
        frs_t = tl([128, T], F32, "frs", 1)
        nc.sync.dma_start(frs_t[:], d["frs_d"].ap())
        mask_t = tl([128, 128], R, "mask", 1)
        nc.sync.dma_start(mask_t[:], d["mask_d"].ap())
        ones_f = tl([128, 64], F32, "onesf", 1)
        nc.gpsimd.memset(ones_f[:], 1.0)
        ones_t = tl([1, 64], F32, "ones", 1)
        nc.vector.tensor_copy(ones_t[:], ones_f[0:1, :])
        eps_t = tl([128, 1], F32, "epsb", 1)
        nc.gpsimd.memset(eps_t[:], EPS)
        ident_t = tl([128, 128], F32, "ident", 1)
        nc.gpsimd.memset(ident_t[:], 0.0)
        nc.gpsimd.affine_select(
            out=ident_t[:], in_=ident_t[:], compare_op=OP.not_equal,
            fill=1.0, base=0, pattern=[[-1, 128]], channel_multiplier=1)

        def transpose_into(chunks_R, src_tiles, off=0):
            """src_tiles: tiles (128 tok, 512 feat) -> chunks_R: NC tiles
            (128 feat, T tok) float32r (rounded via the psum copy)."""
            for tt in range(len(src_tiles)):
                for c in range(NC):
                    pt = pA()
                    nc.tensor.transpose(
                        pt[0:128, 0:128], src_tiles[tt][:, c * 128:(c + 1) * 128],
                        ident_t[:])
                    dst = chunks_R[c][:, (off + tt) * 128:(off + tt + 1) * 128]
                    if (tt + c) % 2 == 0:
                        nc.scalar.copy(dst, pt[0:128, 0:128])
                    else:
                        nc.vector.tensor_copy(dst, pt[0:128, 0:128])

        # ---- load + transpose the two streams ----
        x_tiles = [tl([128, D], F32, "x", 10) for _ in range(NT)]
        y_tiles = [tl([128, D], F32, "y", 2) for _ in range(NT)]
        for tt in range(NT):
            nc.sync.dma_start(x_tiles[tt][:], d["x0_d"].ap()[tt * 128:(tt + 1) * 128, :])
            nc.sync.dma_start(y_tiles[tt][:], d["y0_d"].ap()[tt * 128:(tt + 1) * 128, :])
        xT = [tl([128, T], R, "xT", 8) for _ in range(NC)]
        yT = [tl([128, T], R, "yT", 4) for _ in range(NC)]
        transpose_into(xT, x_tiles)
        transpose_into(yT, y_tiles)

        for li in range(L):
            # ---- layer weights ----
            wk_t = [tl([128, D], R, "wqkv", 12, "wk") for _ in range(NC)]
            wv_t = [tl([128, D], R, "wqkv", 12, "wv") for _ in range(NC)]
            wo_t = [tl([128, D], R, "wqkv", 12, "wo") for _ in range(NC)]
            for k in range(NC):
                nc.sync.dma_start(wk_t[k][:], d["wk_d"].ap()[li, k * 128:(k + 1) * 128, :])
                nc.sync.dma_start(wv_t[k][:], d["wv_d"].ap()[li, k * 128:(k + 1) * 128, :])
                nc.sync.dma_start(wo_t[k][:], d["wo_d"].ap()[li, k * 128:(k + 1) * 128, :])

            bk_t = bv_t = bo_t = b2_t = g1_t = be1_t = g2_t = be2_t = None
            if use_bk:
                bk_t = [tl([128, 1], F32, "bk", 4) for _ in range(NC)]
                for c in range(NC):
                    nc.sync.dma_start(bk_t[c][:], d["bk_d"].ap()[li, c * 128:(c + 1) * 128, :])
            if use_bv:
                bv_t = tl([128, D], F32, "bvb", 1)
                nc.sync.dma_start(bv_t[:], d["bvb_d"].ap()[li])
            if use_bo:
                bo_t = tl([128, D], F32, "bob", 1)
                nc.sync.dma_start(bo_t[:], d["bob_d"].ap()[li])
            if use_b2:
                b2_t = tl([128, D], F32, "b2b", 1)
                nc.sync.dma_start(b2_t[:], d["b2b_d"].ap()[li])
            if use_ln1:
                g1_t = tl([128, D], F32, "g1b", 1)
                be1_t = tl([128, D], F32, "be1b", 1)
                nc.sync.dma_start(g1_t[:], d["g1b_d"].ap()[li])
                nc.sync.dma_start(be1_t[:], d["be1b_d"].ap()[li])
            if use_ln2:
                g2_t = tl([128, D], F32, "g2b", 1)
                be2_t = tl([128, D], F32, "be2b", 1)
                nc.sync.dma_start(g2_t[:], d["g2b_d"].ap()[li])
                nc.sync.dma_start(be2_t[:], d["be2b_d"].ap()[li])

            # ---- V projection -> v_aug tiles (tok, 8*(64 v | 1 one)) ----
            v_aug = [tl([128, 8 * 65], R, "vaug", 8) for _ in range(NT)]
            for tt in range(NT):
                pv = pA()
                for k in range(NC):
                    nc.tensor.matmul(
                        pv[:, 0:512], yT[k][:, tt * 128:(tt + 1) * 128], wv_t[k][:],
                        start=(k == 0), stop=(k == NC - 1))
                vdst = v_aug[tt][:].rearrange("p (g e) -> p g e", e=65)[:, :, 0:64]
                vsrc = pv[:, 0:512].rearrange("p (g e) -> p g e", e=64)
                if use_bv:
                    nc.vector.scalar_tensor_tensor(
                        out=vdst, in0=vsrc, scalar=1.0,
                        in1=bv_t[:].rearrange("p (g e) -> p g e", e=64),
                        op0=OP.mult, op1=OP.add)
                else:
                    nc.scalar.copy(vdst, vsrc)
                nc.vector.tensor_copy(
                    v_aug[tt][:].rearrange("p (g e) -> p g e", e=65)[:, :, 64:65],
                    ones_f[:, 0:8].rearrange("p (g e) -> p g e", e=1))

            # ---- QK projection (per batch half) ----
            # qku[(c, b)]: key side; qks[(c, b)]: query side scaled by frs
            qku = {}
            qks = {}
            for b in range(BPC):
                for c in range(NC):
                    pp = pA()
                    for k in range(NC):
                        nc.tensor.matmul(
                            pp[:, 0:512], wk_t[k][:, c * 128:(c + 1) * 128],
                            xT[k][:, b * 512:(b + 1) * 512],
                            start=(k == 0), stop=(k == NC - 1))
                    u = tl([128, 512], R, "qku", 6)
                    s = tl([128, 512], R, "qks", 6)
                    if use_bk:
                        nc.scalar.activation(u[:], pp[:, 0:512], AF.Identity, bias=bk_t[c][:])
                        nc.vector.scalar_tensor_tensor(
                            out=s[:], in0=pp[:, 0:512], scalar=bk_t[c][:],
                            in1=frs_t[:, b * 512:(b + 1) * 512],
                            op0=OP.add, op1=OP.mult)
                    else:
                        nc.scalar.copy(u[:], pp[:, 0:512])
                        nc.vector.scalar_tensor_tensor(
                            out=s[:], in0=pp[:, 0:512], scalar=1.0,
                            in1=frs_t[:, b * 512:(b + 1) * 512],
                            op0=OP.mult, op1=OP.mult)
                    qku[(c, b)] = u
                    qks[(c, b)] = s

            # ---- attention ----
            uoT = {}
            for b in range(BPC):
                for hp in range(NC):          # head-pair == feature chunk
                    av = [pB(), pB()]         # per parity; row 64 = exp-sums
                    for kb in range(4):
                        n = 512 - 128 * kb
                        pg = [pA(), pA()]
                        for par in range(2):
                            r0 = par * 64
                            nc.tensor.matmul(
                                pg[par][:, 0:n],
                                qku[(hp, b)][r0:r0 + 64, 128 * kb:128 * (kb + 1)],
                                qks[(hp, b)][r0:r0 + 64, 128 * kb:512],
                                start=True, stop=True, tile_position=(r0, 0))
                        for par in range(2):
                            h = 2 * hp + par
                            eT = tl([128, 512], R, "eT", 5)
                            nc.scalar.activation(eT[:, 0:n], pg[par][:, 0:n], AF.Exp)
                            nc.vector.tensor_tensor(
                                eT[:, 0:128], eT[:, 0:128], mask_t[:], OP.mult)
                            nc.tensor.matmul(
                                av[par][0:65, 128 * kb:512],
                                v_aug[b * 4 + kb][:, h * 65:(h + 1) * 65],
                                eT[:, 0:n],
                                start=(kb == 0), stop=(kb == 3),
                                skip_group_check=True)
                    prb = pA()
                    for par in range(2):
                        row = tl([1, 512], F32, "rsb", 3, "rrow")
                        nc.scalar.activation(row[:], av[par][64:65, 0:512],
                                             AF.Copy, bias=1e-30)
                        nc.vector.reciprocal_approx_fast(row[:], row[:])
                        nc.tensor.matmul(prb[par * 64:(par + 1) * 64, 0:512],
                                         ones_t[:], row[:],
                                         start=True, stop=True,
                                         tile_position=(0, par * 64))
                    rb_sb = tl([128, 512], F32, "rbsb", 2)
                    nc.vector.tensor_copy(rb_sb[:], prb[:, 0:512])
                    u = uoT[(hp, b)] = tl([128, 512], R, "uoT", 6)
                    for par in range(2):
                        nc.vector.scalar_tensor_tensor(
                            out=u[par * 64:(par + 1) * 64, :],
                            in0=av[par][0:64, 0:512], scalar=1.0,
                            in1=rb_sb[par * 64:(par + 1) * 64, :],
                            op0=OP.mult, op1=OP.mult)


            # ---- O projection + residual + LN1 ----
            x_mid = [tl([128, D], F32, "x", 10, "xmid") for _ in range(NT)]
            for tt in range(NT):
                po = pA()
                for c in range(NC):
                    nc.tensor.matmul(
                        po[:, 0:512],
                        uoT[(c, tt // 4)][:, (tt % 4) * 128:(tt % 4 + 1) * 128],
                        wo_t[c][:],
                        start=(c == 0), stop=(c == NC - 1))
                if use_bo:
                    nc.vector.tensor_tensor(po[:, 0:512], po[:, 0:512], bo_t[:], OP.add)
                _layernorm(nc, tl, po[:, 0:512], x_tiles[tt], x_mid[tt],
                           (g1_t, be1_t) if use_ln1 else None, eps_t)
            xTm = [tl([128, T], R, "xT", 8, "xTm") for _ in range(NC)]
            transpose_into(xTm, x_mid)

            # ---- FFN (two token-half passes) + residual + LN2 ----
            x_out = [None] * NT
            for half in range(2):
                accs = [pB() for _ in range(4)]
                for f in range(NF):
                    w1f = tl([128, 512], R, "w1f", 4)
                    for k in range(NC):
                        nc.sync.dma_start(
                            w1f[:, k * 128:(k + 1) * 128],
                            d["w1_d"].ap()[li, k * 128:(k + 1) * 128, f * 128:(f + 1) * 128])
                    ph = pA()
                    for k in range(NC):
                        nc.tensor.matmul(
                            ph[:, 0:512], w1f[:, k * 128:(k + 1) * 128],
                            xTm[k][:, half * 512:(half + 1) * 512],
                            start=(k == 0), stop=(k == NC - 1))
                    hf = tl([128, 512], R, "hf", 3)
                    if use_b1:
                        b1f = tl([128, 1], F32, "b1f", 3)
                        nc.sync.dma_start(b1f[:], d["b1_d"].ap()[li, f * 128:(f + 1) * 128, :])
                        nc.scalar.activation(hf[:], ph[:, 0:512], AF.Relu, bias=b1f[:])
                    else:
                        nc.scalar.activation(hf[:], ph[:, 0:512], AF.Relu)
                    w2f = tl([128, 512], R, "w2f", 4)
                    nc.sync.dma_start(w2f[:], d["w2_d"].ap()[li, f * 128:(f + 1) * 128, :])
                    for q in range(4):
                        nc.tensor.matmul(
                            accs[q][:, 0:512], hf[:, q * 128:(q + 1) * 128], w2f[:],
                            start=(f == 0), stop=(f == NF - 1))
                for q in range(4):
                    tt = half * 4 + q
                    if use_b2:
                        nc.vector.tensor_tensor(accs[q][:, 0:512], accs[q][:, 0:512],
                                                b2_t[:], OP.add)
                    x_out[tt] = tl([128, D], F32, "x", 10, "xout")
                    _layernorm(nc, tl, accs[q][:, 0:512], x_mid[tt], x_out[tt],
                               (g2_t, be2_t) if use_ln2 else None, eps_t)

            if li < L - 1:
                xT = [tl([128, T], R, "xT", 8, "xTn") for _ in range(NC)]
                transpose_into(xT, x_out)
                x_tiles = x_out
            else:
                for tt in range(NT):
                    nc.sync.dma_start(d["out_d"].ap()[tt * 128:(tt + 1) * 128, :],
                                      x_out[tt][:])


def _layernorm(nc, tl, p_in, x_old, x_new, gb, eps_t):
    """x_new = LN(p_in + x_old) over the free dim (512), fp32.
    p_in: psum AP (128,512); x_old/x_new sbuf (128,512) fp32."""
    xres = tl([128, D], F32, "xres", 3)
    msum = tl([128, 1], F32, "lnst", 14, "msum")
    nc.vector.scalar_tensor_tensor(
        out=xres[:], in0=p_in, scalar=1.0, in1=x_old[:],
        op0=OP.mult, op1=OP.add, accum_out=msum[:])
    scr = tl([128, D], F32, "lnscr", 2)
    sqsum = tl([128, 1], F32, "lnst", 14, "sqsum")
    nc.scalar.activation(scr[:], xres[:], AF.Square, accum_out=sqsum[:])
    mu = tl([128, 1], F32, "lnst", 14, "mu")
    nc.vector.tensor_scalar_mul(mu[:], msum[:], 1.0 / D)
    # var = E[x^2] - mu^2 = (mu * -mu) + e2  -> via (mu*mu - e2) * -1
    var = tl([128, 1], F32, "lnst", 14, "var")
    nc.vector.tensor_scalar(
        out=var[:], in0=sqsum[:], scalar1=1.0 / D, scalar2=None, op0=OP.mult)
    nc.vector.scalar_tensor_tensor(
        out=var[:], in0=mu[:], scalar=mu[:], in1=var[:], op0=OP.mult,
        op1=OP.subtract)
    nc.vector.tensor_scalar_mul(var[:], var[:], -1.0)
    sd = tl([128, 1], F32, "lnst", 14, "sd")
    nc.scalar.activation(sd[:], var[:], AF.Sqrt, bias=eps_t[:])
    rstd = tl([128, 1], F32, "lnst", 14, "rstd")
    nc.vector.reciprocal(rstd[:], sd[:])
    negmu = tl([128, 1], F32, "lnst", 14, "negmu")
    nc.vector.tensor_scalar_mul(negmu[:], mu[:], -1.0)
    if gb is None:
        nc.vector.tensor_scalar(
            out=x_new[:], in0=xres[:], scalar1=negmu[:], scalar2=rstd[:],
            op0=OP.add, op1=OP.mult)
    else:
        g_t, be_t = gb
        xn = tl([128, D], F32, "xn", 2)
        nc.vector.tensor_scalar(
            out=xn[:], in0=xres[:], scalar1=negmu[:], scalar2=rstd[:],
            op0=OP.add, op1=OP.mult)
        nc.vector.tensor_tensor(xn[:], xn[:], g_t[:], OP.mult)
        nc.vector.tensor_tensor(x_new[:], xn[:], be_t[:], OP.add)


def _host_prep(inputs):
    q = np.asarray(inputs["q_embed"], np.float32)
    qa = np.asarray(inputs["qa_embed"], np.float32)
    fr = np.asarray(inputs["forget_rate"], np.float32)
    pe = np.asarray(inputs["pe"], np.float32)
    x0 = q + pe
    y0 = qa + pe

    flags = (
        bool(np.any(inputs["bk"])), bool(np.any(inputs["bv"])),
        bool(np.any(inputs["bo"])), bool(np.any(inputs["b1"])),
        bool(np.any(inputs["b2"])),
        bool(np.any(np.asarray(inputs["ln1_g"]) != 1.0) or np.any(inputs["ln1_b"])),
        bool(np.any(np.asarray(inputs["ln2_g"]) != 1.0) or np.any(inputs["ln2_b"])),
    )

    mask01 = (np.arange(128)[None, :] > np.arange(128)[:, None]).astype(np.float32)

    def bcast(v):  # (L, D) -> (L, 128, D)
        v = np.asarray(v, np.float32)
        return np.ascontiguousarray(np.broadcast_to(v[:, None, :], (L, 128, v.shape[-1])))

    common = {
        "Wk": np.ascontiguousarray(inputs["Wk"], np.float32),
        "Wv": np.ascontiguousarray(inputs["Wv"], np.float32),
        "Wo": np.ascontiguousarray(inputs["Wo"], np.float32),
        "W1": np.ascontiguousarray(inputs["W1"], np.float32),
        "W2": np.ascontiguousarray(inputs["W2"], np.float32),
        "bk": np.ascontiguousarray(inputs["bk"], np.float32).reshape(L, D, 1),
        "b1": np.ascontiguousarray(inputs["b1"], np.float32).reshape(L, DFF, 1),
        "bvb": bcast(inputs["bv"]), "bob": bcast(inputs["bo"]),
        "b2b": bcast(inputs["b2"]),
        "g1b": bcast(inputs["ln1_g"]), "be1b": bcast(inputs["ln1_b"]),
        "g2b": bcast(inputs["ln2_g"]), "be2b": bcast(inputs["ln2_b"]),
        "mask01": mask01,
    }

    in_maps = []
    for c in range(NCORES):
        sl = slice(c * BPC, (c + 1) * BPC)
        frs = (fr[sl, :, 0].reshape(1, T) * SCALE).astype(np.float32)
        m = dict(common)
        m["x0"] = np.ascontiguousarray(x0[sl].reshape(T, D))
        m["y0"] = np.ascontiguousarray(y0[sl].reshape(T, D))
        m["frs"] = np.ascontiguousarray(np.broadcast_to(frs, (128, T)))
        in_maps.append(m)
    return in_maps, flags


def kernel(_trace=False, **inputs):
    in_maps, flags = _host_prep(inputs)
    if flags not in _CACHE:
        _CACHE[flags] = _build(flags)
    nc = _CACHE[flags]
    br = run_bass_kernel_spmd(nc, in_maps, list(range(NCORES)), trace=_trace)
    out = np.empty((B, S, D), np.float32)
    for c in range(NCORES):
        out[c * BPC:(c + 1) * BPC] = br.results[c]["out"].reshape(BPC, S, D)
    if _trace:
        kernel.last_result = br
    return out



# revision 14
# speedup vs baseline: 1.1075x; 1.1075x over previous
"""DeepBKT 4-layer transformer forward on 8 TRN2 NeuronCores.

Data-parallel over batch: B=16 -> 2 batch items per core; each core runs the
full 4-layer stack on its (2*512, 512) token slab.

v2 (bf16): all matmul operands bf16 (PE full rate, cheap LDWEIGHTS), fp32
residual/LayerNorm.  Weights are host-converted to bf16 and DMA'd in
coalesced chunks.  Attention keeps the transposed-score layout with the
forget-rate folded into the query copy; softmax denominators come from a
ones-column appended to V; the reciprocal row is broadcast across the 64
partitions of each head parity with a single (2,128)-stationary bf16 matmul
per head-pair.  Plain psum->sbuf copies run on the (otherwise idle) gpsimd
engine; transposes are batched 4-to-a-psum-bank so one gpsimd copy drains
them.  Score/exp/AV are software-pipelined so the scalar engine's exp
stream paces the PE with minimal idle.
"""

import numpy as np
import ml_dtypes

import concourse.bass as bass
import concourse.tile as tile
from concourse import bacc, mybir
from concourse.bass_utils import run_bass_kernel_spmd

F32 = mybir.dt.float32
BF = mybir.dt.bfloat16
AF = mybir.ActivationFunctionType
OP = mybir.AluOpType

B, S, D, H, DFF, L = 16, 512, 512, 8, 2048, 4
DK = D // H                       # 64
NCORES = 8
BPC = B // NCORES                 # 2 batch items per core
T = BPC * S                       # 1024 tokens per core
NT = T // 128                     # 8 token tiles
NC = D // 128                     # 4 feature chunks
NF = DFF // 128                   # 16 ffn chunks
EPS = 1e-5
SCALE = 1.0 / np.sqrt(DK)

_CACHE = {}


def _build(flags):
    nc = bacc.Bacc("TRN2", target_bir_lowering=False, debug=False,
                   num_devices=NCORES)

    d = {}
    d["x0_d"] = nc.dram_tensor("x0", [T, D], F32, kind="ExternalInput")
    d["y0_d"] = nc.dram_tensor("y0", [T, D], F32, kind="ExternalInput")
    d["frs_d"] = nc.dram_tensor("frs", [128, T], F32, kind="ExternalInput")
    d["mask_d"] = nc.dram_tensor("mask01", [128, 128], BF, kind="ExternalInput")
    # wqkv: per layer (128, k-chunk 4, 512) with [p, k, j] = W[k*128+p, j]
    d["wk_d"] = nc.dram_tensor("Wk", [L, 128, NC * D], BF, kind="ExternalInput")
    d["wv_d"] = nc.dram_tensor("Wv", [L, 128, NC * D], BF, kind="ExternalInput")
    d["wo_d"] = nc.dram_tensor("Wo", [L, 128, NC * D], BF, kind="ExternalInput")
    # W1: (L*NF, 128, 4*128) with [li*NF+f, p, k*128+j] = W1[li, k*128+p, f*128+j]
    d["w1_d"] = nc.dram_tensor("W1", [L * NF, 128, D], BF, kind="ExternalInput")
    # W2: (L*NF, 128, 512) direct slices
    d["w2_d"] = nc.dram_tensor("W2", [L * NF, 128, D], BF, kind="ExternalInput")
    d["bk_d"] = nc.dram_tensor("bk", [L, D, 1], F32, kind="ExternalInput")
    d["b1_d"] = nc.dram_tensor("b1", [L, DFF, 1], F32, kind="ExternalInput")
    # free-axis vectors pre-broadcast on host to (128, D)
    for nm in ("bvb", "bob", "b2b", "g1b", "be1b", "g2b", "be2b"):
        d[nm + "_d"] = nc.dram_tensor(nm, [L, 128, D], F32, kind="ExternalInput")
    d["out_d"] = nc.dram_tensor("out", [T, D], F32, kind="ExternalOutput")

    with tile.TileContext(nc) as tc:
        _emit(nc, tc, d, flags)
    nc.compile()
    return nc


def _emit(nc, tc, d, flags):
    use_bk, use_bv, use_bo, use_b1, use_b2, use_ln1, use_ln2 = flags
    import contextlib
    ctx = contextlib.ExitStack()
    with ctx:
        sb = ctx.enter_context(tc.tile_pool(name="sb", bufs=1))
        ps = ctx.enter_context(tc.tile_pool(name="ps", bufs=4, space="PSUM"))

        def tl(shape, dtype, tag, bufs, name=None):
            return sb.tile(shape, dtype, tag=tag, bufs=bufs, name=name or tag)

        def pA():
            return ps.tile([128, 512], F32, tag="psA", bufs=4, name="psA")

        def pB():
            return ps.tile([128, 512], F32, tag="psB", bufs=4, name="psB")

        # ---- constants ----
        frs_t = tl([128, T], F32, "frs", 1)
        nc.sync.dma_start(frs_t[:], d["frs_d"].ap())
        mask_t = tl([128, 128], BF, "mask", 1)
        nc.sync.dma_start(mask_t[:], d["mask_d"].ap())
        ones1_t = tl([1, 64], BF, "ones1", 1)
        nc.gpsimd.memset(ones1_t[:], 1.0)
        eps_t = tl([128, 1], F32, "epsb", 1)
        nc.gpsimd.memset(eps_t[:], EPS)
        identb = tl([128, 128], F32, "ident", 1)
        nc.gpsimd.memset(identb[:], 0.0)
        nc.gpsimd.affine_select(
            out=identb[:], in_=identb[:], compare_op=OP.not_equal,
            fill=1.0, base=0, pattern=[[-1, 128]], channel_multiplier=1)

        def transpose_into(dst_all, src_tiles, off=0):
            """src_tiles: NT tiles (128 tok, 512 feat) fp32 -> dst_all: one
            tile (128, NC*T) bf16 laid out [p, c, tok].  4 transposes fill one
            psum bank; one gpsimd copy drains it (with f32->bf16 cast)."""
            dview = dst_all[:].rearrange("p (c t) -> p c t", t=T)
            for tt in range(len(src_tiles)):
                pt = pA()
                for c in range(NC):
                    nc.tensor.transpose(
                        pt[:, c * 128:(c + 1) * 128],
                        src_tiles[tt][:, c * 128:(c + 1) * 128],
                        identb[:])
                dst = dview[:, :, (off + tt) * 128:(off + tt + 1) * 128]
                src = pt[:, 0:512].rearrange("p (c t) -> p c t", t=128)
                if tt % 2 == 0:
                    nc.scalar.copy(dst, src)
                else:
                    nc.vector.tensor_copy(dst, src)

        # ---- load + transpose the two streams ----
        x_tiles = [tl([128, D], F32, "x", 12) for _ in range(NT)]
        y_tiles = [tl([128, D], F32, "y", 2) for _ in range(NT)]
        for tt in range(NT):
            nc.sync.dma_start(x_tiles[tt][:], d["x0_d"].ap()[tt * 128:(tt + 1) * 128, :])
            nc.sync.dma_start(y_tiles[tt][:], d["y0_d"].ap()[tt * 128:(tt + 1) * 128, :])
        xT = tl([128, NC * T], BF, "xT", 3, "xT0")
        yT = tl([128, NC * T], BF, "yT", 1)
        transpose_into(xT, x_tiles)
        transpose_into(yT, y_tiles)

        def xs(xt, k, lo, hi):
            """moving slice of a transposed stream: chunk k, tokens lo:hi."""
            return xt[:, k * T + lo:k * T + hi]

        for li in range(L):
            # ---- layer weights (coalesced bf16 loads) ----
            wk_t = tl([128, NC * D], BF, "wqkv", 4, "wk")
            wv_t = tl([128, NC * D], BF, "wqkv", 4, "wv")
            wo_t = tl([128, NC * D], BF, "wqkv", 4, "wo")
            nc.sync.dma_start(wk_t[:], d["wk_d"].ap()[li])
            nc.sync.dma_start(wv_t[:], d["wv_d"].ap()[li])
            nc.sync.dma_start(wo_t[:], d["wo_d"].ap()[li])
            w1_t = [tl([128, D], BF, "w1", 16, "w1") for _ in range(NF)]
            w2_t = [tl([128, D], BF, "w2", 16, "w2") for _ in range(NF)]
            for f in range(NF):
                nc.sync.dma_start(w1_t[f][:], d["w1_d"].ap()[li * NF + f])
                nc.sync.dma_start(w2_t[f][:], d["w2_d"].ap()[li * NF + f])

            bk_t = bv_t = bo_t = b2_t = g1_t = be1_t = g2_t = be2_t = None
            if use_bk:
                bk_t = [tl([128, 1], F32, "bk", 8) for _ in range(NC)]
                for c in range(NC):
                    nc.sync.dma_start(bk_t[c][:], d["bk_d"].ap()[li, c * 128:(c + 1) * 128, :])
            if use_bv:
                bv_t = tl([128, D], F32, "bvb", 2)
                nc.sync.dma_start(bv_t[:], d["bvb_d"].ap()[li])
            if use_bo:
                bo_t = tl([128, D], F32, "bob", 2)
                nc.sync.dma_start(bo_t[:], d["bob_d"].ap()[li])
            if use_b2:
                b2_t = tl([128, D], F32, "b2b", 2)
                nc.sync.dma_start(b2_t[:], d["b2b_d"].ap()[li])
            if use_ln1:
                g1_t = tl([128, D], F32, "g1b", 2)
                be1_t = tl([128, D], F32, "be1b", 2)
                nc.sync.dma_start(g1_t[:], d["g1b_d"].ap()[li])
                nc.sync.dma_start(be1_t[:], d["be1b_d"].ap()[li])
            if use_ln2:
                g2_t = tl([128, D], F32, "g2b", 2)
                be2_t = tl([128, D], F32, "be2b", 2)
                nc.sync.dma_start(g2_t[:], d["g2b_d"].ap()[li])
                nc.sync.dma_start(be2_t[:], d["be2b_d"].ap()[li])

            # ---- V projection -> v_aug tiles (tok, 8*(64 v | 1 one)) ----
            v_aug = [tl([128, 8 * 65], BF, "vaug", 10) for _ in range(NT)]
            for tt in range(NT):
                pv = pA()
                for k in range(NC):
                    nc.tensor.matmul(
                        pv[:, 0:512], xs(yT, k, tt * 128, (tt + 1) * 128),
                        wv_t[:, k * D:(k + 1) * D],
                        start=(k == 0), stop=(k == NC - 1))
                vdst = v_aug[tt][:].rearrange("p (g e) -> p g e", e=65)[:, :, 0:64]
                vsrc = pv[:, 0:512].rearrange("p (g e) -> p g e", e=64)
                if use_bv:
                    nc.vector.scalar_tensor_tensor(
                        out=vdst, in0=vsrc, scalar=1.0,
                        in1=bv_t[:].rearrange("p (g e) -> p g e", e=64),
                        op0=OP.mult, op1=OP.add)
                else:
                    nc.scalar.copy(vdst, vsrc)
                nc.gpsimd.memset(
                    v_aug[tt][:].rearrange("p (g e) -> p g e", e=65)[:, :, 64:65], 1.0)

            # ---- QK projection + attention + O-proj + LN1, per batch item --
            x_mid = [None] * NT
            xTm = tl([128, NC * T], BF, "xT", 3, "xTm")
            for b in range(BPC):
                qku = {}
                qks = {}
                for c in range(NC):
                    pp = pA()
                    for k in range(NC):
                        nc.tensor.matmul(
                            pp[:, 0:512], wk_t[:, k * D + c * 128:k * D + (c + 1) * 128],
                            xs(xT, k, b * 512, (b + 1) * 512),
                            start=(k == 0), stop=(k == NC - 1))
                    u = tl([128, 512], BF, "qku", 10)
                    s = tl([128, 512], BF, "qks", 10)
                    if use_bk:
                        nc.scalar.activation(u[:], pp[:, 0:512], AF.Identity, bias=bk_t[c][:])
                        nc.vector.scalar_tensor_tensor(
                            out=s[:], in0=pp[:, 0:512], scalar=bk_t[c][:],
                            in1=frs_t[:, b * 512:(b + 1) * 512],
                            op0=OP.add, op1=OP.mult)
                    else:
                        nc.scalar.copy(u[:], pp[:, 0:512])
                        nc.vector.scalar_tensor_tensor(
                            out=s[:], in0=pp[:, 0:512], scalar=1.0,
                            in1=frs_t[:, b * 512:(b + 1) * 512],
                            op0=OP.mult, op1=OP.mult)
                    qku[c] = u
                    qks[c] = s

                # attention, software-pipelined over head-pair groups
                uoT = {}

                def emit_denorm(avs, hp):
                    rows = tl([1, 1024], F32, "rows", 3)
                    for par in range(2):
                        nc.vector.tensor_scalar_add(
                            rows[0:1, par * 512:(par + 1) * 512],
                            avs[par][64:65, 0:512], 1e-30)
                    nc.vector.reciprocal(rows[:], rows[:])
                    rb16 = tl([1, 1024], BF, "rb16", 3)
                    nc.gpsimd.tensor_copy(rb16[:], rows[:])
                    prb = pA()
                    for par in range(2):
                        nc.tensor.matmul(
                            prb[par * 64:(par + 1) * 64, 0:512], ones1_t[:],
                            rb16[0:1, par * 512:(par + 1) * 512],
                            start=True, stop=True, tile_position=(0, par * 64))
                    rb_sb = tl([128, 512], BF, "rbsb", 3)
                    if hp % 2 == 0:
                        nc.scalar.copy(rb_sb[:], prb[:, 0:512])
                    else:
                        nc.vector.tensor_copy(rb_sb[:], prb[:, 0:512])
                    u = uoT[hp] = tl([128, 512], BF, "uoT", 10)
                    for par in range(2):
                        nc.vector.tensor_tensor(
                            u[par * 64:(par + 1) * 64, :],
                            avs[par][0:64, 0:512],
                            rb_sb[par * 64:(par + 1) * 64, :],
                            OP.mult)

                prev = None
                for hp in range(NC):
                    qu, qs = qku[hp], qks[hp]
                    pgs = {}

                    def score(kb):
                        n = 512 - 128 * kb
                        pgs[kb] = [pA(), pA()]
                        for par in range(2):
                            r0 = par * 64
                            nc.tensor.matmul(
                                pgs[kb][par][:, 0:n],
                                qu[r0:r0 + 64, 128 * kb:128 * (kb + 1)],
                                qs[r0:r0 + 64, 128 * kb:512],
                                start=True, stop=True, tile_position=(r0, 0))

                    score(0)
                    score(1)
                    if prev is not None:
                        emit_denorm(*prev)
                    avs = [pB(), pB()]
                    for kb in range(4):
                        n = 512 - 128 * kb
                        eTs = []
                        for par in range(2):
                            eT = tl([128, 512], BF, "eT", 6)
                            nc.scalar.activation(eT[:, 0:n], pgs[kb][par][:, 0:n], AF.Exp)
                            nc.gpsimd.tensor_tensor(
                                eT[:, 0:128], eT[:, 0:128], mask_t[:], OP.mult)
                            eTs.append(eT)
                        if kb + 2 < 4:
                            score(kb + 2)
                        for par in range(2):
                            h = 2 * hp + par
                            nc.tensor.matmul(
                                avs[par][0:65, 128 * kb:512],
                                v_aug[b * 4 + kb][:, h * 65:(h + 1) * 65],
                                eTs[par][:, 0:n],
                                start=(kb == 0), stop=(kb == 3),
                                skip_group_check=True)
                    prev = (avs, hp)
                emit_denorm(*prev)

                # ---- O projection + residual + LN1 for this batch item ----
                for q in range(4):
                    tt = b * 4 + q
                    po = pA()
                    for c in range(NC):
                        nc.tensor.matmul(
                            po[:, 0:512],
                            uoT[c][:, q * 128:(q + 1) * 128],
                            wo_t[:, c * D:(c + 1) * D],
                            start=(c == 0), stop=(c == NC - 1))
                    if use_bo:
                        nc.vector.tensor_tensor(po[:, 0:512], po[:, 0:512], bo_t[:], OP.add)
                    x_mid[tt] = tl([128, D], F32, "x", 12, "xmid")
                    _layernorm(nc, tl, po[:, 0:512], x_tiles[tt], x_mid[tt],
                               (g1_t, be1_t) if use_ln1 else None, eps_t)
                transpose_into(xTm, x_mid[b * 4:(b + 1) * 4], off=b * 4)

            # ---- FFN (two token-half passes, sw-pipelined) + LN2 ----
            x_out = [None] * NT
            for half in range(2):
                accs = [pB() for _ in range(4)]
                pending = None

                def emit_w2(pend):
                    f, hf = pend
                    for q in range(4):
                        nc.tensor.matmul(
                            accs[q][:, 0:512], hf[:, q * 128:(q + 1) * 128], w2_t[f][:],
                            start=(f == 0), stop=(f == NF - 1),
                            skip_group_check=True)

                for f in range(NF):
                    ph = pA()
                    for k in range(NC):
                        nc.tensor.matmul(
                            ph[:, 0:512], w1_t[f][:, k * 128:(k + 1) * 128],
                            xs(xTm, k, half * 512, (half + 1) * 512),
                            start=(k == 0), stop=(k == NC - 1))
                    hf = tl([128, 512], BF, "hf", 3)
                    if use_b1:
                        b1f = tl([128, 1], F32, "b1f", 3)
                        nc.sync.dma_start(b1f[:], d["b1_d"].ap()[li, f * 128:(f + 1) * 128, :])
                        nc.scalar.activation(hf[:], ph[:, 0:512], AF.Relu, bias=b1f[:])
                    else:
                        nc.scalar.activation(hf[:], ph[:, 0:512], AF.Relu)
                    if pending is not None:
                        emit_w2(pending)
                    pending = (f, hf)
                emit_w2(pending)
                for q in range(4):
                    tt = half * 4 + q
                    if use_b2:
                        nc.vector.tensor_tensor(accs[q][:, 0:512], accs[q][:, 0:512],
                                                b2_t[:], OP.add)
                    x_out[tt] = tl([128, D], F32, "x", 12, "xout")
                    _layernorm(nc, tl, accs[q][:, 0:512], x_mid[tt], x_out[tt],
                               (g2_t, be2_t) if use_ln2 else None, eps_t)

            if li < L - 1:
                xT = tl([128, NC * T], BF, "xT", 3, "xTn")
                transpose_into(xT, x_out)
                x_tiles = x_out
            else:
                for tt in range(NT):
                    nc.sync.dma_start(d["out_d"].ap()[tt * 128:(tt + 1) * 128, :],
                                      x_out[tt][:])


def _layernorm(nc, tl, p_in, x_old, x_new, gb, eps_t):
    """x_new = LN(p_in + x_old) over the free dim (512), fp32.
    p_in: psum AP (128,512); x_old/x_new sbuf (128,512) fp32."""
    xres = tl([128, D], F32, "xres", 3)
    msum = tl([128, 1], F32, "lnst", 14, "msum")
    nc.vector.scalar_tensor_tensor(
        out=xres[:], in0=p_in, scalar=1.0, in1=x_old[:],
        op0=OP.mult, op1=OP.add, accum_out=msum[:])
    scr = tl([128, D], F32, "lnscr", 2)
    sqsum = tl([128, 1], F32, "lnst", 14, "sqsum")
    nc.scalar.activation(scr[:], xres[:], AF.Square, accum_out=sqsum[:])
    mu = tl([128, 1], F32, "lnst", 14, "mu")
    nc.vector.tensor_scalar_mul(mu[:], msum[:], 1.0 / D)
    # var = E[x^2] - mu^2 = -(mu*mu - E[x^2])
    var = tl([128, 1], F32, "lnst", 14, "var")
    nc.vector.tensor_scalar(
        out=var[:], in0=sqsum[:], scalar1=1.0 / D, scalar2=None, op0=OP.mult)
    nc.vector.scalar_tensor_tensor(
        out=var[:], in0=mu[:], scalar=mu[:], in1=var[:], op0=OP.mult,
        op1=OP.subtract)
    nc.vector.tensor_scalar_mul(var[:], var[:], -1.0)
    sd = tl([128, 1], F32, "lnst", 14, "sd")
    nc.scalar.activation(sd[:], var[:], AF.Sqrt, bias=eps_t[:])
    rstd = tl([128, 1], F32, "lnst", 14, "rstd")
    nc.vector.reciprocal(rstd[:], sd[:])
    negmu = tl([128, 1], F32, "lnst", 14, "negmu")
    nc.vector.tensor_scalar_mul(negmu[:], mu[:], -1.0)
    if gb is None:
        nc.vector.tensor_scalar(
            out=x_new[:], in0=xres[:], scalar1=negmu[:], scalar2=rstd[:],
            op0=OP.add, op1=OP.mult)
    else:
        g_t, be_t = gb
        xn = tl([128, D], F32, "xn", 2)
        nc.vector.tensor_scalar(
            out=xn[:], in0=xres[:], scalar1=negmu[:], scalar2=rstd[:],
            op0=OP.add, op1=OP.mult)
        nc.vector.tensor_tensor(xn[:], xn[:], g_t[:], OP.mult)
        nc.vector.tensor_tensor(x_new[:], xn[:], be_t[:], OP.add)


def _host_prep(inputs):
    bf16 = ml_dtypes.bfloat16
    q = np.asarray(inputs["q_embed"], np.float32)
    qa = np.asarray(inputs["qa_embed"], np.float32)
    fr = np.asarray(inputs["forget_rate"], np.float32)
    pe = np.asarray(inputs["pe"], np.float32)
    x0 = q + pe
    y0 = qa + pe

    flags = (
        bool(np.any(inputs["bk"])), bool(np.any(inputs["bv"])),
        bool(np.any(inputs["bo"])), bool(np.any(inputs["b1"])),
        bool(np.any(inputs["b2"])),
        bool(np.any(np.asarray(inputs["ln1_g"]) != 1.0) or np.any(inputs["ln1_b"])),
        bool(np.any(np.asarray(inputs["ln2_g"]) != 1.0) or np.any(inputs["ln2_b"])),
    )

    mask01 = (np.arange(128)[None, :] > np.arange(128)[:, None]).astype(bf16)

    def wqkv(w):  # (L, D, D) -> (L, 128, 4*D) bf16, [li, p, k*D+j] = W[li, k*128+p, j]
        w = np.asarray(w, np.float32).reshape(L, NC, 128, D)
        return np.ascontiguousarray(
            w.transpose(0, 2, 1, 3).reshape(L, 128, NC * D)).astype(bf16)

    w1 = np.asarray(inputs["W1"], np.float32).reshape(L, NC, 128, NF, 128)
    w1 = np.ascontiguousarray(
        w1.transpose(0, 3, 2, 1, 4).reshape(L * NF, 128, NC * 128)).astype(bf16)
    w2 = np.ascontiguousarray(
        np.asarray(inputs["W2"], np.float32).reshape(L * NF, 128, D)).astype(bf16)

    def bcast(v):  # (L, D) -> (L, 128, D)
        v = np.asarray(v, np.float32)
        return np.ascontiguousarray(np.broadcast_to(v[:, None, :], (L, 128, v.shape[-1])))

    common = {
        "Wk": wqkv(inputs["Wk"]),
        "Wv": wqkv(inputs["Wv"]),
        "Wo": wqkv(inputs["Wo"]),
        "W1": w1,
        "W2": w2,
        "bk": np.ascontiguousarray(inputs["bk"], np.float32).reshape(L, D, 1),
        "b1": np.ascontiguousarray(inputs["b1"], np.float32).reshape(L, DFF, 1),
        "bvb": bcast(inputs["bv"]), "bob": bcast(inputs["bo"]),
        "b2b": bcast(inputs["b2"]),
        "g1b": bcast(inputs["ln1_g"]), "be1b": bcast(inputs["ln1_b"]),
        "g2b": bcast(inputs["ln2_g"]), "be2b": bcast(inputs["ln2_b"]),
        "mask01": mask01,
    }

    in_maps = []
    for c in range(NCORES):
        sl = slice(c * BPC, (c + 1) * BPC)
        frs = (fr[sl, :, 0].reshape(1, T) * SCALE).astype(np.float32)
        m = dict(common)
        m["x0"] = np.ascontiguousarray(x0[sl].reshape(T, D))
        m["y0"] = np.ascontiguousarray(y0[sl].reshape(T, D))
        m["frs"] = np.ascontiguousarray(np.broadcast_to(frs, (128, T)))
        in_maps.append(m)
    return in_maps, flags


def kernel(_trace=False, **inputs):
    in_maps, flags = _host_prep(inputs)
    if flags not in _CACHE:
        _CACHE[flags] = _build(flags)
    nc = _CACHE[flags]
    br = run_bass_kernel_spmd(nc, in_maps, list(range(NCORES)), trace=_trace)
    out = np.empty((B, S, D), np.float32)
    for c in range(NCORES):
        out[c * BPC:(c + 1) * BPC] = br.results[c]["out"].reshape(BPC, S, D)
    if _trace:
        kernel.last_result = br
    return out


# revision 19
# speedup vs baseline: 1.4855x; 1.3413x over previous
"""DeepBKT 4-layer transformer forward on 8 TRN2 NeuronCores.

Data-parallel over batch: B=16 -> 2 batch items per core; each core runs the
full 4-layer stack on its (2*512, 512) token slab.

v2 (bf16): all matmul operands bf16 (PE full rate, cheap LDWEIGHTS), fp32
residual/LayerNorm.  Weights are host-converted to bf16 and DMA'd in
coalesced chunks.  Attention keeps the transposed-score layout with the
forget-rate folded into the query copy; softmax denominators come from a
ones-column appended to V; the reciprocal row is broadcast across the 64
partitions of each head parity with a single (2,128)-stationary bf16 matmul
per head-pair.  Plain psum->sbuf copies run on the (otherwise idle) gpsimd
engine; transposes are batched 4-to-a-psum-bank so one gpsimd copy drains
them.  Score/exp/AV are software-pipelined so the scalar engine's exp
stream paces the PE with minimal idle.
"""

import numpy as np
import ml_dtypes

import concourse.bass as bass
import concourse.tile as tile
from concourse import bacc, mybir
from concourse.bass_utils import run_bass_kernel_spmd

F32 = mybir.dt.float32
BF = mybir.dt.bfloat16
AF = mybir.ActivationFunctionType
OP = mybir.AluOpType

B, S, D, H, DFF, L = 16, 512, 512, 8, 2048, 4
DK = D // H                       # 64
NCORES = 8
BPC = B // NCORES                 # 2 batch items per core
T = BPC * S                       # 1024 tokens per core
NT = T // 128                     # 8 token tiles
NC = D // 128                     # 4 feature chunks
NF = DFF // 128                   # 16 ffn chunks
EPS = 1e-5
SCALE = 1.0 / np.sqrt(DK)

_CACHE = {}


def _build(flags):
    nc = bacc.Bacc("TRN2", target_bir_lowering=False, debug=False,
                   num_devices=NCORES)

    d = {}
    d["x0_d"] = nc.dram_tensor("x0", [T, D], F32, kind="ExternalInput")
    d["y0_d"] = nc.dram_tensor("y0", [T, D], F32, kind="ExternalInput")
    d["frs_d"] = nc.dram_tensor("frs", [128, T], F32, kind="ExternalInput")
    d["mask_d"] = nc.dram_tensor("mask01", [128, 128], BF, kind="ExternalInput")
    # wqkv: per layer (128, k-chunk 4, 512) with [p, k, j] = W[k*128+p, j]
    d["wk_d"] = nc.dram_tensor("Wk", [L, 128, NC * D], BF, kind="ExternalInput")
    d["wv_d"] = nc.dram_tensor("Wv", [L, 128, NC * D], BF, kind="ExternalInput")
    d["wo_d"] = nc.dram_tensor("Wo", [L, 128, NC * D], BF, kind="ExternalInput")
    # W1: (L*NF, 128, 4*128) with [li*NF+f, p, k*128+j] = W1[li, k*128+p, f*128+j]
    d["w1_d"] = nc.dram_tensor("W1", [L * NF, 128, D], BF, kind="ExternalInput")
    # W2: (L*NF, 128, 512) direct slices
    d["w2_d"] = nc.dram_tensor("W2", [L * NF, 128, D], BF, kind="ExternalInput")
    d["bk_d"] = nc.dram_tensor("bk", [L, D, 1], F32, kind="ExternalInput")
    d["b1_d"] = nc.dram_tensor("b1", [L, DFF, 1], F32, kind="ExternalInput")
    # free-axis vectors pre-broadcast on host to (128, D)
    for nm in ("bvb", "bob", "b2b", "g1b", "be1b", "g2b", "be2b"):
        d[nm + "_d"] = nc.dram_tensor(nm, [L, 128, D], F32, kind="ExternalInput")
    d["out_d"] = nc.dram_tensor("out", [T, D], F32, kind="ExternalOutput")

    with tile.TileContext(nc) as tc:
        _emit(nc, tc, d, flags)
    nc.compile()
    return nc


def _emit(nc, tc, d, flags):
    use_bk, use_bv, use_bo, use_b1, use_b2, use_ln1, use_ln2 = flags
    import contextlib
    ctx = contextlib.ExitStack()
    with ctx:
        sb = ctx.enter_context(tc.tile_pool(name="sb", bufs=1))
        ps = ctx.enter_context(tc.tile_pool(name="ps", bufs=4, space="PSUM"))

        def tl(shape, dtype, tag, bufs, name=None):
            return sb.tile(shape, dtype, tag=tag, bufs=bufs, name=name or tag)

        def pA():
            return ps.tile([128, 512], F32, tag="psA", bufs=4, name="psA")

        def pB():
            return ps.tile([128, 512], F32, tag="psB", bufs=4, name="psB")

        # ---- constants ----
        frs_t = tl([128, T], F32, "frs", 1)
        nc.sync.dma_start(frs_t[:], d["frs_d"].ap())
        mask_t = tl([128, 128], BF, "mask", 1)
        nc.sync.dma_start(mask_t[:], d["mask_d"].ap())
        ones1_t = tl([1, 64], BF, "ones1", 1)
        nc.gpsimd.memset(ones1_t[:], 1.0)
        eps_t = tl([128, 1], F32, "epsb", 1)
        nc.gpsimd.memset(eps_t[:], EPS)
        identb = tl([128, 128], F32, "ident", 1)
        nc.gpsimd.memset(identb[:], 0.0)
        nc.gpsimd.affine_select(
            out=identb[:], in_=identb[:], compare_op=OP.not_equal,
            fill=1.0, base=0, pattern=[[-1, 128]], channel_multiplier=1)

        def transpose_into(dst_all, src_tiles, off=0):
            """src_tiles: NT tiles (128 tok, 512 feat) fp32 -> dst_all: one
            tile (128, NC*T) bf16 laid out [p, c, tok].  4 transposes fill one
            psum bank; one gpsimd copy drains it (with f32->bf16 cast)."""
            dview = dst_all[:].rearrange("p (c t) -> p c t", t=T)
            for tt in range(len(src_tiles)):
                pt = pA()
                for c in range(NC):
                    nc.tensor.transpose(
                        pt[:, c * 128:(c + 1) * 128],
                        src_tiles[tt][:, c * 128:(c + 1) * 128],
                        identb[:])
                dst = dview[:, :, (off + tt) * 128:(off + tt + 1) * 128]
                src = pt[:, 0:512].rearrange("p (c t) -> p c t", t=128)
                if tt % 2 == 0:
                    nc.scalar.copy(dst, src)
                else:
                    nc.vector.tensor_copy(dst, src)

        # ---- load + transpose the two streams ----
        x_tiles = [tl([128, D], F32, "x", 12) for _ in range(NT)]
        y_tiles = [tl([128, D], F32, "y", 2) for _ in range(NT)]
        for tt in range(NT):
            nc.sync.dma_start(x_tiles[tt][:], d["x0_d"].ap()[tt * 128:(tt + 1) * 128, :])
            nc.sync.dma_start(y_tiles[tt][:], d["y0_d"].ap()[tt * 128:(tt + 1) * 128, :])
        xT = tl([128, NC * T], BF, "xT", 3, "xT0")
        yT = tl([128, NC * T], BF, "yT", 1)
        transpose_into(xT, x_tiles)
        transpose_into(yT, y_tiles)

        def xs(xt, k, lo, hi):
            """moving slice of a transposed stream: chunk k, tokens lo:hi."""
            return xt[:, k * T + lo:k * T + hi]

        for li in range(L):
            # ---- layer weights (coalesced bf16 loads) ----
            wk_t = tl([128, NC * D], BF, "wqkv", 4, "wk")
            wv_t = tl([128, NC * D], BF, "wqkv", 4, "wv")
            wo_t = tl([128, NC * D], BF, "wqkv", 4, "wo")
            nc.sync.dma_start(wk_t[:], d["wk_d"].ap()[li])
            nc.sync.dma_start(wv_t[:], d["wv_d"].ap()[li])
            nc.sync.dma_start(wo_t[:], d["wo_d"].ap()[li])
            w1_t = [tl([128, D], BF, "w1", 16, "w1") for _ in range(NF)]
            w2_t = [tl([128, D], BF, "w2", 16, "w2") for _ in range(NF)]
            for f in range(NF):
                nc.sync.dma_start(w1_t[f][:], d["w1_d"].ap()[li * NF + f])
                nc.sync.dma_start(w2_t[f][:], d["w2_d"].ap()[li * NF + f])

            bk_t = bv_t = bo_t = b2_t = g1_t = be1_t = g2_t = be2_t = None
            if use_bk:
                bk_t = [tl([128, 1], F32, "bk", 8) for _ in range(NC)]
                for c in range(NC):
                    nc.sync.dma_start(bk_t[c][:], d["bk_d"].ap()[li, c * 128:(c + 1) * 128, :])
            if use_bv:
                bv_t = tl([128, D], F32, "bvb", 2)
                nc.sync.dma_start(bv_t[:], d["bvb_d"].ap()[li])
            if use_bo:
                bo_t = tl([128, D], F32, "bob", 2)
                nc.sync.dma_start(bo_t[:], d["bob_d"].ap()[li])
            if use_b2:
                b2_t = tl([128, D], F32, "b2b", 2)
                nc.sync.dma_start(b2_t[:], d["b2b_d"].ap()[li])
            if use_ln1:
                g1_t = tl([128, D], F32, "g1b", 2)
                be1_t = tl([128, D], F32, "be1b", 2)
                nc.sync.dma_start(g1_t[:], d["g1b_d"].ap()[li])
                nc.sync.dma_start(be1_t[:], d["be1b_d"].ap()[li])
            if use_ln2:
                g2_t = tl([128, D], F32, "g2b", 2)
                be2_t = tl([128, D], F32, "be2b", 2)
                nc.sync.dma_start(g2_t[:], d["g2b_d"].ap()[li])
                nc.sync.dma_start(be2_t[:], d["be2b_d"].ap()[li])

            # ---- V projection -> v_aug tiles (tok, 8*(64 v | 1 one)) ----
            v_aug = [tl([128, 8 * 65], BF, "vaug", 10) for _ in range(NT)]
            for tt in range(NT):
                pv = pA()
                for k in range(NC):
                    nc.tensor.matmul(
                        pv[:, 0:512], xs(yT, k, tt * 128, (tt + 1) * 128),
                        wv_t[:, k * D:(k + 1) * D],
                        start=(k == 0), stop=(k == NC - 1))
                vdst = v_aug[tt][:].rearrange("p (g e) -> p g e", e=65)[:, :, 0:64]
                vsrc = pv[:, 0:512].rearrange("p (g e) -> p g e", e=64)
                if use_bv:
                    nc.vector.scalar_tensor_tensor(
                        out=vdst, in0=vsrc, scalar=1.0,
                        in1=bv_t[:].rearrange("p (g e) -> p g e", e=64),
                        op0=OP.mult, op1=OP.add)
                else:
                    nc.scalar.copy(vdst, vsrc)
                nc.gpsimd.memset(
                    v_aug[tt][:].rearrange("p (g e) -> p g e", e=65)[:, :, 64:65], 1.0)

            # ---- QK projection + attention + O-proj + LN1, per batch item --
            x_mid = [None] * NT
            xTm = tl([128, NC * T], BF, "xT", 3, "xTm")
            for b in range(BPC):
                qku = {}
                qks = {}
                for c in range(NC):
                    pp = pA()
                    for k in range(NC):
                        nc.tensor.matmul(
                            pp[:, 0:512], wk_t[:, k * D + c * 128:k * D + (c + 1) * 128],
                            xs(xT, k, b * 512, (b + 1) * 512),
                            start=(k == 0), stop=(k == NC - 1))
                    u = tl([128, 512], BF, "qku", 10)
                    s = tl([128, 512], BF, "qks", 10)
                    if use_bk:
                        nc.scalar.activation(u[:], pp[:, 0:512], AF.Identity, bias=bk_t[c][:])
                        nc.vector.scalar_tensor_tensor(
                            out=s[:], in0=pp[:, 0:512], scalar=bk_t[c][:],
                            in1=frs_t[:, b * 512:(b + 1) * 512],
                            op0=OP.add, op1=OP.mult)
                    else:
                        nc.scalar.copy(u[:], pp[:, 0:512])
                        nc.vector.scalar_tensor_tensor(
                            out=s[:], in0=pp[:, 0:512], scalar=1.0,
                            in1=frs_t[:, b * 512:(b + 1) * 512],
                            op0=OP.mult, op1=OP.mult)
                    qku[c] = u
                    qks[c] = s

                # attention, software-pipelined over head-pair groups
                uoT = {}

                def emit_denorm(avs, hp):
                    rows = tl([1, 1024], F32, "rows", 3)
                    for par in range(2):
                        nc.scalar.activation(
                            rows[0:1, par * 512:(par + 1) * 512],
                            avs[par][64:65, 0:512], AF.Copy, bias=1e-30)
                    nc.vector.reciprocal_approx_fast(rows[:], rows[:])
                    rb16 = tl([1, 1024], BF, "rb16", 3)
                    nc.vector.tensor_copy(rb16[:], rows[:])
                    prb = pA()
                    for par in range(2):
                        nc.tensor.matmul(
                            prb[par * 64:(par + 1) * 64, 0:512], ones1_t[:],
                            rb16[0:1, par * 512:(par + 1) * 512],
                            start=True, stop=True, tile_position=(0, par * 64))
                    rb_sb = tl([128, 512], BF, "rbsb", 3)
                    if hp % 2 == 0:
                        nc.scalar.copy(rb_sb[:], prb[:, 0:512])
                    else:
                        nc.vector.tensor_copy(rb_sb[:], prb[:, 0:512])
                    u = uoT[hp] = tl([128, 512], BF, "uoT", 10)
                    for par in range(2):
                        nc.vector.tensor_tensor(
                            u[par * 64:(par + 1) * 64, :],
                            avs[par][0:64, 0:512],
                            rb_sb[par * 64:(par + 1) * 64, :],
                            OP.mult)

                prev = None
                for hp in range(NC):
                    qu, qs = qku[hp], qks[hp]
                    pgs = {}

                    def score(kb):
                        n = 512 - 128 * kb
                        pgs[kb] = [pA(), pA()]
                        for par in range(2):
                            r0 = par * 64
                            nc.tensor.matmul(
                                pgs[kb][par][:, 0:n],
                                qu[r0:r0 + 64, 128 * kb:128 * (kb + 1)],
                                qs[r0:r0 + 64, 128 * kb:512],
                                start=True, stop=True, tile_position=(r0, 0))

                    score(0)
                    score(1)
                    if prev is not None:
                        emit_denorm(*prev)
                    avs = [pB(), pB()]
                    for kb in range(4):
                        n = 512 - 128 * kb
                        eTs = []
                        for par in range(2):
                            eT = tl([128, 512], BF, "eT", 6)
                            nc.scalar.activation(eT[:, 0:n], pgs[kb][par][:, 0:n], AF.Exp)
                            nc.vector.tensor_tensor(
                                eT[:, 0:128], eT[:, 0:128], mask_t[:], OP.mult)
                            eTs.append(eT)
                        if kb + 2 < 4:
                            score(kb + 2)
                        for par in range(2):
                            h = 2 * hp + par
                            nc.tensor.matmul(
                                avs[par][0:65, 128 * kb:512],
                                v_aug[b * 4 + kb][:, h * 65:(h + 1) * 65],
                                eTs[par][:, 0:n],
                                start=(kb == 0), stop=(kb == 3),
                                skip_group_check=True)
                    prev = (avs, hp)
                emit_denorm(*prev)

                # ---- O projection + residual + LN1 for this batch item ----
                for q in range(4):
                    tt = b * 4 + q
                    po = pA()
                    for c in range(NC):
                        nc.tensor.matmul(
                            po[:, 0:512],
                            uoT[c][:, q * 128:(q + 1) * 128],
                            wo_t[:, c * D:(c + 1) * D],
                            start=(c == 0), stop=(c == NC - 1))
                    if use_bo:
                        nc.vector.tensor_tensor(po[:, 0:512], po[:, 0:512], bo_t[:], OP.add)
                    x_mid[tt] = tl([128, D], F32, "x", 12, "xmid")
                    _layernorm(nc, tl, po[:, 0:512], x_tiles[tt], x_mid[tt],
                               (g1_t, be1_t) if use_ln1 else None, eps_t)
                transpose_into(xTm, x_mid[b * 4:(b + 1) * 4], off=b * 4)

            # ---- FFN (two token-half passes, sw-pipelined) + LN2 ----
            x_out = [None] * NT
            for half in range(2):
                accs = [pB() for _ in range(4)]
                pending = None

                def emit_w2(pend):
                    f, hf = pend
                    for q in range(4):
                        nc.tensor.matmul(
                            accs[q][:, 0:512], hf[:, q * 128:(q + 1) * 128], w2_t[f][:],
                            start=(f == 0), stop=(f == NF - 1),
                            skip_group_check=True)

                for f in range(NF):
                    ph = pA()
                    for k in range(NC):
                        nc.tensor.matmul(
                            ph[:, 0:512], w1_t[f][:, k * 128:(k + 1) * 128],
                            xs(xTm, k, half * 512, (half + 1) * 512),
                            start=(k == 0), stop=(k == NC - 1))
                    hf = tl([128, 512], BF, "hf", 3)
                    if use_b1:
                        b1f = tl([128, 1], F32, "b1f", 3)
                        nc.sync.dma_start(b1f[:], d["b1_d"].ap()[li, f * 128:(f + 1) * 128, :])
                        nc.scalar.activation(hf[:], ph[:, 0:512], AF.Relu, bias=b1f[:])
                    else:
                        nc.scalar.activation(hf[:], ph[:, 0:512], AF.Relu)
                    if pending is not None:
                        emit_w2(pending)
                    pending = (f, hf)
                emit_w2(pending)
                for q in range(4):
                    tt = half * 4 + q
                    if use_b2:
                        nc.vector.tensor_tensor(accs[q][:, 0:512], accs[q][:, 0:512],
                                                b2_t[:], OP.add)
                    x_out[tt] = tl([128, D], F32, "x", 12, "xout")
                    _layernorm(nc, tl, accs[q][:, 0:512], x_mid[tt], x_out[tt],
                               (g2_t, be2_t) if use_ln2 else None, eps_t)

            if li < L - 1:
                xT = tl([128, NC * T], BF, "xT", 3, "xTn")
                transpose_into(xT, x_out)
                x_tiles = x_out
            else:
                for tt in range(NT):
                    nc.sync.dma_start(d["out_d"].ap()[tt * 128:(tt + 1) * 128, :],
                                      x_out[tt][:])


def _layernorm(nc, tl, p_in, x_old, x_new, gb, eps_t):
    """x_new = LN(p_in + x_old) over the free dim (512), fp32.
    p_in: psum AP (128,512); x_old/x_new sbuf (128,512) fp32."""
    xres = tl([128, D], F32, "xres", 3)
    msum = tl([128, 1], F32, "lnst", 14, "msum")
    nc.vector.scalar_tensor_tensor(
        out=xres[:], in0=p_in, scalar=1.0, in1=x_old[:],
        op0=OP.mult, op1=OP.add, accum_out=msum[:])
    scr = tl([128, D], F32, "lnscr", 2)
    sqsum = tl([128, 1], F32, "lnst", 14, "sqsum")
    nc.scalar.activation(scr[:], xres[:], AF.Square, accum_out=sqsum[:])
    mu = tl([128, 1], F32, "lnst", 14, "mu")
    nc.vector.tensor_scalar_mul(mu[:], msum[:], 1.0 / D)
    # var = E[x^2] - mu^2 = -(mu*mu - E[x^2])
    var = tl([128, 1], F32, "lnst", 14, "var")
    nc.vector.tensor_scalar(
        out=var[:], in0=sqsum[:], scalar1=1.0 / D, scalar2=None, op0=OP.mult)
    nc.vector.scalar_tensor_tensor(
        out=var[:], in0=mu[:], scalar=mu[:], in1=var[:], op0=OP.mult,
        op1=OP.subtract)
    nc.vector.tensor_scalar_mul(var[:], var[:], -1.0)
    sd = tl([128, 1], F32, "lnst", 14, "sd")
    nc.scalar.activation(sd[:], var[:], AF.Sqrt, bias=eps_t[:])
    rstd = tl([128, 1], F32, "lnst", 14, "rstd")
    nc.vector.reciprocal(rstd[:], sd[:])
    negmu = tl([128, 1], F32, "lnst", 14, "negmu")
    nc.vector.tensor_scalar_mul(negmu[:], mu[:], -1.0)
    if gb is None:
        nc.vector.tensor_scalar(
            out=x_new[:], in0=xres[:], scalar1=negmu[:], scalar2=rstd[:],
            op0=OP.add, op1=OP.mult)
    else:
        g_t, be_t = gb
        xn = tl([128, D], F32, "xn", 2)
        nc.vector.tensor_scalar(
            out=xn[:], in0=xres[:], scalar1=negmu[:], scalar2=rstd[:],
            op0=OP.add, op1=OP.mult)
        nc.vector.tensor_tensor(xn[:], xn[:], g_t[:], OP.mult)
        nc.vector.tensor_tensor(x_new[:], xn[:], be_t[:], OP.add)


def _host_prep(inputs):
    bf16 = ml_dtypes.bfloat16
    q = np.asarray(inputs["q_embed"], np.float32)
    qa = np.asarray(inputs["qa_embed"], np.float32)
    fr = np.asarray(inputs["forget_rate"], np.float32)
    pe = np.asarray(inputs["pe"], np.float32)
    x0 = q + pe
    y0 = qa + pe

    flags = (
        bool(np.any(inputs["bk"])), bool(np.any(inputs["bv"])),
        bool(np.any(inputs["bo"])), bool(np.any(inputs["b1"])),
        bool(np.any(inputs["b2"])),
        bool(np.any(np.asarray(inputs["ln1_g"]) != 1.0) or np.any(inputs["ln1_b"])),
        bool(np.any(np.asarray(inputs["ln2_g"]) != 1.0) or np.any(inputs["ln2_b"])),
    )

    mask01 = (np.arange(128)[None, :] > np.arange(128)[:, None]).astype(bf16)

    def wqkv(w):  # (L, D, D) -> (L, 128, 4*D) bf16, [li, p, k*D+j] = W[li, k*128+p, j]
        w = np.asarray(w, np.float32).reshape(L, NC, 128, D)
        return np.ascontiguousarray(
            w.transpose(0, 2, 1, 3).reshape(L, 128, NC * D)).astype(bf16)

    w1 = np.asarray(inputs["W1"], np.float32).reshape(L, NC, 128, NF, 128)
    w1 = np.ascontiguousarray(
        w1.transpose(0, 3, 2, 1, 4).reshape(L * NF, 128, NC * 128)).astype(bf16)
    w2 = np.ascontiguousarray(
        np.asarray(inputs["W2"], np.float32).reshape(L * NF, 128, D)).astype(bf16)

    def bcast(v):  # (L, D) -> (L, 128, D)
        v = np.asarray(v, np.float32)
        return np.ascontiguousarray(np.broadcast_to(v[:, None, :], (L, 128, v.shape[-1])))

    common = {
        "Wk": wqkv(inputs["Wk"]),
        "Wv": wqkv(inputs["Wv"]),
        "Wo": wqkv(inputs["Wo"]),
        "W1": w1,
        "W2": w2,
        "bk": np.ascontiguousarray(inputs["bk"], np.float32).reshape(L, D, 1),
        "b1": np.ascontiguousarray(inputs["b1"], np.float32).reshape(L, DFF, 1),
        "bvb": bcast(inputs["bv"]), "bob": bcast(inputs["bo"]),
        "b2b": bcast(inputs["b2"]),
        "g1b": bcast(inputs["ln1_g"]), "be1b": bcast(inputs["ln1_b"]),
        "g2b": bcast(inputs["ln2_g"]), "be2b": bcast(inputs["ln2_b"]),
        "mask01": mask01,
    }

    in_maps = []
    for c in range(NCORES):
        sl = slice(c * BPC, (c + 1) * BPC)
        frs = (fr[sl, :, 0].reshape(1, T) * SCALE).astype(np.float32)
        m = dict(common)
        m["x0"] = np.ascontiguousarray(x0[sl].reshape(T, D))
        m["y0"] = np.ascontiguousarray(y0[sl].reshape(T, D))
        m["frs"] = np.ascontiguousarray(np.broadcast_to(frs, (128, T)))
        in_maps.append(m)
    return in_maps, flags


def kernel(_trace=False, **inputs):
    in_maps, flags = _host_prep(inputs)
    if flags not in _CACHE:
        _CACHE[flags] = _build(flags)
    nc = _CACHE[flags]
    br = run_bass_kernel_spmd(nc, in_maps, list(range(NCORES)), trace=_trace)
    out = np.empty((B, S, D), np.float32)
    for c in range(NCORES):
        out[c * BPC:(c + 1) * BPC] = br.results[c]["out"].reshape(BPC, S, D)
    if _trace:
        kernel.last_result = br
    return out
